# revision 50
# baseline (speedup 1.0000x reference)
"""DenseCapsule routing kernel for Trainium2 (Bass/Tile), 8-core data-parallel.

Problem: x [64, 8192, 8], W [8, 160], bias [160] ->
  x_hat = (x @ W + bias).reshape(64, 8192, 10, 16)
  3 dynamic-routing iterations (softmax over out_num=10, weighted sum over
  in_num=8192, squash over the 10-axis, agreement update), return
  ||outputs||_2 over out_dim -> [64, 10].

Key algebra (x_hat never materialized):
  yT[(b,d), (j,b')] = sum_i x_aug[i,(b,d)] c[i,(j,b')]   (PE, masked by cBLK)
  s8T[k, (j,b)]     = per-j matmuls W_aug vs masked yT    (PE, f32)
  squash runs on the k-partition layout [16, (j,b)] so the vhat matmuls
  need no transposes; vT[d, (j,b)] via per-j matmuls; blkv = mask(cREP@vacc).
  b_logits = xT^T @ blkv, single fp16 blkv (no hi/lo split).
  softmax: exp on ACT (bf16), Z pair-tree on DVE 2x, c = e*Zr -> fp16.

Sharding: batch 64 -> 8 cores x 8 batches. Row space (b,d) = b*9+d (72 rows),
cols (j,b) = j*8+b (80). Output row [1, 80] = lengths at (j,b).
"""

from contextlib import ExitStack

import numpy as np

import concourse.bacc as bacc
import concourse.bass as bass
import concourse.mybir as mybir
import concourse.tile as tile
import concourse.bass_utils as bass_utils

f32 = mybir.dt.float32
bf16 = mybir.dt.bfloat16
fp16 = mybir.dt.float16
AF = mybir.ActivationFunctionType
ALU = mybir.AluOpType

P = 128          # SBUF partitions
NH = 64          # i-chunks per batch (8192 / 128)
NB = 8           # batches per core
D = 8            # input capsule dim
DA = 9           # augmented (+ ones column)
J = 10           # out_num
KD = 16          # out_dim
KT = NB * DA     # 72 rows (b, d)
BJ = NB * J      # 80 cols (j, b) = j*8+b
IN = 8192
N_CORES = 8


def _build_nc():
    nc = bacc.Bacc(
        "TRN2", target_bir_lowering=False, debug=False, num_devices=N_CORES
    )

    xf_d = nc.dram_tensor(
        "xf", [4, P, NH // 4, KT], fp16, kind="ExternalInput"
    ).ap()
    xt_d = nc.dram_tensor(
        "xt", [4, KT, NH // 4, P], fp16, kind="ExternalInput"
    ).ap()
    w_d = nc.dram_tensor("W", [D, J * KD], f32, kind="ExternalInput").ap()
    bias_d = nc.dram_tensor("bias", [J * KD], f32, kind="ExternalInput").ap()
    out_d = nc.dram_tensor("out", [1, BJ], f32, kind="ExternalOutput").ap()

    # ---- structural constants ----
    # cpack cols: 0:80 cBLK (rows 0:72), 80:152 cREP (rows 0:9),
    #             152:161 eye9 (rows 0:9), 161:162 ones column (all rows)
    C_BLK, C_REP, C_E9, C_ONE = 0, 80, 152, 161
    CPW = C_ONE + 1
    cpack_np = np.zeros((P, CPW), dtype=np.float32)
    for b in range(NB):
        for d in range(DA):
            for j in range(J):
                cpack_np[b * DA + d, C_BLK + j * NB + b] = 1.0
    for d in range(DA):
        for b in range(NB):
            cpack_np[d, C_REP + b * DA + d] = 1.0
    cpack_np[0:DA, C_E9:C_E9 + DA] = np.eye(DA, dtype=np.float32)
    cpack_np[:, C_ONE] = 1.0

    cpack_d = nc.inline_tensor(cpack_np, "cpack").ap()

    with tile.TileContext(nc) as tc, ExitStack() as ctx:
        sbp = ctx.enter_context(tc.tile_pool(name="sbp", bufs=1))

        def T(shape, name, dt=f32):
            return sbp.tile(shape, dt, name=name, tag=name)

        # ----- persistent SBUF tensors -----
        x_f16 = T([P, NH, KT], "x_f16", fp16)         # fp16 x_aug (host-cast)
        xT = T([KT, NH, P], "xT", fp16)               # x_aug^T (host-built)
        cpack = T([P, CPW], "cpack")
        cBLK = cpack[0:KT, C_BLK:C_BLK + BJ]
        cREP = cpack[0:DA, C_REP:C_REP + KT]
        eye9 = cpack[0:DA, C_E9:C_E9 + DA]
        onesF = cpack[:, C_ONE:C_ONE + 1]

        W10flat = T([DA, J * KD], "W10flat")          # W_aug rows d
        WBIGall = T([KT, J * KD], "WBIGall")          # W_aug repl. over b
        WT10 = T([KD, J, DA], "WT10")                 # W_aug^T per j
        part0 = T([P, 4, KT], "part0")                # m0 f32 colsum partials
        y0row = T([1, KT], "y0row")                   # m0 colsum row
        tenth80 = T([1, BJ], "tenth80")               # 0.1 expander row
        phalf16 = T([KD, NB], "phalf16")              # +0.5 (pow -> sqrt)
        halfrow = T([1, BJ], "halfrow")               # +0.5 (pow warm)
        vaccT = T([DA, BJ], "vaccT")                  # accumulated vhat^T
        blkv_t = [T([KT, BJ], f"blkv{m}", fp16) for m in range(2)]
        lsum = T([1, BJ], "lsum")
        powwarm = T([1, 1], "powwarm")

        e_st = [None, T([P, NH, J, NB], "e1", bf16), T([P, NH, J, NB], "e2", bf16)]
        c_st = [None, T([P, NH, J, NB], "c1", fp16), T([P, NH, J, NB], "c2", fp16)]
        u5_t = [None, T([P, NH, 5, NB], "u5_1", bf16), T([P, NH, 5, NB], "u5_2", bf16)]
        v2_t = [None, T([P, NH, 2, NB], "v2_1", bf16), T([P, NH, 2, NB], "v2_2", bf16)]
        w1_t = [None, T([P, NH, NB], "w1_1", bf16), T([P, NH, NB], "w1_2", bf16)]
        Z_t = [None, T([P, NH, NB], "Z_1"), T([P, NH, NB], "Z_2")]
        Zr_t = [None, T([P, NH, NB], "Zr_1"), T([P, NH, NB], "Zr_2")]
        Zrb_t = [None, T([P, NH, NB], "Zrb_1", bf16), T([P, NH, NB], "Zrb_2", bf16)]

        # per-m tail tensors
        yTm_t = [T([KT, BJ], f"yTm_{m}") for m in range(3)]
        s2T_t = [T([KD, J + 1, NB], f"s2T_{m}") for m in range(3)]
        nsq_t = [T([KD, NB], f"nsq_{m}") for m in range(3)]
        uin_t = [T([KD, NB], f"uin_{m}") for m in range(3)]
        wp1_t = [T([KD, NB], f"wp1_{m}") for m in range(3)]
        r1_t = [T([KD, NB], f"r1_{m}") for m in range(3)]
        sc_t = [T([KD, NB], f"sc_{m}") for m in range(3)]
        o8T_t = [T([KD, J, NB], f"o8T_{m}") for m in range(3)]
        osqT = T([KD, J, NB], "osqT")

        # ----- inputs: xf pieces first on sync HWDGE (feeds colsums);
        # consts + xt split over scalar HWDGE and gpsimd SWDGE ----------
        for k in range(4):
            nc.sync.dma_start(
                x_f16[:, k * (NH // 4):(k + 1) * (NH // 4), :], xf_d[k]
            )
        nc.scalar.dma_start(cpack[:, :], cpack_d[:, :])
        nc.scalar.dma_start(W10flat[0:D, :], w_d[:, :])
        nc.scalar.dma_start(
            W10flat[D:DA, :],
            bass.AP(tensor=bias_d.tensor, offset=0,
                    ap=[[J * KD, 1], [1, J * KD]]),
        )
        for k in (0, 1):
            nc.scalar.dma_start(
                xT[:, k * (NH // 4):(k + 1) * (NH // 4), :], xt_d[k]
            )
        for k in (2, 3):
            nc.gpsimd.dma_start(
                xT[:, k * (NH // 4):(k + 1) * (NH // 4), :], xt_d[k]
            )

        # tiny memsets on gpsimd; pad memset on DVE (runs during first DMAs)
        nc.gpsimd.memset(phalf16[:, :], 0.5)
        nc.gpsimd.memset(halfrow[:, :], 0.5)
        nc.gpsimd.memset(tenth80[:, :], 1.0 / J)
        for m in range(3):
            nc.gpsimd.memset(s2T_t[m][:, J, :], 1e-12)
        # warm the gpsimd pow library off the critical chain
        nc.gpsimd.tensor_tensor(
            powwarm[:, :], halfrow[0:1, 0:1], halfrow[0:1, 0:1], ALU.pow
        )

        # ----- m0 colsums (DVE): per-piece partials over h, then merge ----
        for k in range(4):
            nc.vector.reduce_sum(
                part0[:, k, :],
                x_f16[:, k * (NH // 4):(k + 1) * (NH // 4), :]
                .transpose([0, 2, 1]),
                axis=mybir.AxisListType.X,
            )

        with tc.tile_pool(name="wpp", bufs=1, space="PSUM") as wpp:
            # ---- W prep: WBIGall + WT10 (early, overlaps x DMA) ----
            wb_ps = wpp.tile([KT, J * KD], f32, tag="wb", name="wb_ps")
            nc.tensor.matmul(
                wb_ps[:, :], cREP, W10flat[:, :], start=True, stop=True
            )
            nc.scalar.copy(WBIGall[:, :], wb_ps[:, :])
            wt_ps = wpp.tile([KD, J, DA], f32, tag="wt", name="wt_ps")
            for j in range(J):
                nc.tensor.transpose(
                    wt_ps[:, j, :], W10flat[:, j * KD:(j + 1) * KD], eye9
                )
            nc.scalar.copy(WT10[:, :, :], wt_ps[:, :, :])



        with tc.tile_pool(name="ypp", bufs=1, space="PSUM") as ypp:

            def tail(m, yT72, fill=()):
                """mask -> s-MMs -> squash (k-layout) -> v-MMs -> blkv.

                fill: callbacks emitting PE work injected after the s-MM /
                v-MM stages so PE stays busy during the DVE scalar chain.
                """
                yTm, s2T = yTm_t[m], s2T_t[m]
                nsqT, u, wp1, r1, scT = (
                    nsq_t[m], uin_t[m], wp1_t[m], r1_t[m], sc_t[m]
                )
                o8T = o8T_t[m]
                # mask: yTm[(b,d),(j,b')] = yT * (b==b')
                nc.vector.tensor_tensor(yTm[:, :], yT72, cBLK, ALU.mult)
                s8_ps = ypp.tile([KD, J, NB], f32, tag="ysm", name=f"s8_{m}")
                for j in range(J):
                    nc.tensor.matmul(
                        s8_ps[:, j, :],
                        WBIGall[:, j * KD:(j + 1) * KD],
                        yTm[:, j * NB:(j + 1) * NB],
                        start=True, stop=True,
                    )

                # squash scalars on [16, 8] (k-partition layout);
                # eps lives in s2T's 11th j-column (prologue memset)
                nc.scalar.activation(s2T[:, 0:J, :], s8_ps[:, :, :], AF.Square)
                nc.vector.reduce_sum(
                    nsqT[:, :],
                    s2T[:, :, :].transpose([0, 2, 1]),
                    axis=mybir.AxisListType.X,
                )
                # sc = sqrt(nsq)/(1+nsq): pow on Pool overlaps the DVE recip
                nc.gpsimd.tensor_tensor(
                    u[:, :], nsqT[:, :], phalf16[:, :], ALU.pow
                )
                nc.vector.tensor_scalar_add(wp1[:, :], nsqT[:, :], 1.0)
                nc.vector.reciprocal_approx_fast(r1[:, :], wp1[:, :])
                if len(fill) > 0:
                    fill[0]()
                nc.vector.tensor_tensor(scT[:, :], u[:, :], r1[:, :], ALU.mult)
                # o8T = s8 * sc, straight from PSUM (sc broadcast over j)
                nc.vector.tensor_tensor(
                    o8T[:, :, :],
                    s8_ps[:, :, :],
                    scT[:, :].unsqueeze(1).broadcast_to((KD, J, NB)),
                    ALU.mult,
                )
                if m == 2:
                    # final lengths: ||o||_k per (j,b) via ones-matmul
                    nc.scalar.activation(
                        osqT[:, :, :], o8T[:, :, :], AF.Square
                    )
                    ls_ps = ypp.tile([1, BJ], f32, tag="ysm", name="ls_ps")
                    nc.tensor.matmul(
                        ls_ps[:, :], onesF[0:KD, :],
                        osqT[:, :, :].rearrange("p j b -> p (j b)"),
                        start=True, stop=True,
                    )
                    nc.vector.tensor_copy(lsum[:, :], ls_ps[:, :])
                    nc.sync.dma_start(out_d[:, :], lsum[:, :])
                    return
                # vhat: vT[d,(j,b)] via per-j matmuls; accumulate; expand+mask
                vT_ps = ypp.tile([DA, BJ], f32, tag="ysm", name=f"vT_{m}")
                for j in range(J):
                    nc.tensor.matmul(
                        vT_ps[:, j * NB:(j + 1) * NB],
                        WT10[:, j, :],
                        o8T[:, j, :],
                        start=True, stop=True,
                    )
                if len(fill) > 1:
                    fill[1]()
                if m == 0:
                    nc.vector.tensor_copy(vaccT[:, :], vT_ps[:, :])
                else:
                    nc.vector.tensor_tensor(
                        vaccT[:, :], vaccT[:, :], vT_ps[:, :], ALU.add
                    )
                vd_ps = ypp.tile([KT, BJ], f32, tag="ysm", name=f"vd_{m}")
                nc.tensor.matmul(
                    vd_ps[:, :], cREP, vaccT[:, :], start=True, stop=True
                )
                nc.vector.tensor_tensor(
                    blkv_t[m][:, :], vd_ps[:, :], cBLK, ALU.mult
                )

            # ================= m = 0 (uniform c shortcut, f32) =================
            # accumulate the 4 piece-partials in PSUM via 4 tiny matmuls
            y0r_ps = ypp.tile([1, KT], f32, tag="ysm", name="y0r")
            for k in range(4):
                nc.tensor.matmul(
                    y0r_ps[:, :], onesF, part0[:, k, :],
                    start=(k == 0), stop=(k == 3),
                )
            nc.vector.tensor_copy(y0row[:, :], y0r_ps[:, :])
            yT0_ps = ypp.tile([KT, BJ], f32, tag="ysm", name="yT0")
            nc.tensor.matmul(
                yT0_ps[:, :], y0row[:, :], tenth80[:, :], start=True, stop=True
            )

            tail(0, yT0_ps[:, :])

            # ================= m = 1, 2 =================
            with tc.tile_pool(name="bwp", bufs=3, space="PSUM") as bwp:
                for m in (1, 2):
                    blkv = blkv_t[m - 1]
                    e = e_st[m]
                    cst = c_st[m]
                    u5, v2, w1 = u5_t[m], v2_t[m], w1_t[m]
                    Z, Zr, Zrb = Z_t[m], Zr_t[m], Zrb_t[m]

                    yT_ps = ypp.tile([KT, BJ], f32, tag="ysm", name=f"yT_{m}")

                    def emit_logits_exp(h0, QH):
                        # waves of 8 chunks; exp per wave
                        for w in range(QH // 8):
                            hw0 = h0 + w * 8
                            bw = bwp.tile(
                                [P, 2, 512], f32, tag="bw",
                                name=f"bw_{m}_{hw0}",
                            )
                            for c8 in range(8):
                                h = hw0 + c8
                                off = (c8 % 4) * BJ
                                nc.tensor.matmul(
                                    bw[:, c8 // 4, off:off + BJ],
                                    xT[:, h, :], blkv[:, :],
                                    start=True, stop=True,
                                )
                            nc.scalar.activation(
                                e[:, hw0:hw0 + 8, :, :]
                                .rearrange("p (a c) j b -> p a c (j b)", a=2),
                                bw[:, :, 0:4 * BJ]
                                .rearrange("p a (c x) -> p a c x", x=BJ),
                                AF.Exp,
                            )

                    def emit_tree_c_y(h0, QH, last=False):
                        hs = slice(h0, h0 + QH)
                        nc.vector.tensor_tensor(
                            u5[:, hs, :, :], e[:, hs, 0:5, :], e[:, hs, 5:10, :],
                            ALU.add,
                        )
                        nc.vector.tensor_tensor(
                            v2[:, hs, :, :], u5[:, hs, 0:2, :], u5[:, hs, 2:4, :],
                            ALU.add,
                        )
                        nc.vector.tensor_tensor(
                            w1[:, hs, :], v2[:, hs, 0, :], v2[:, hs, 1, :],
                            ALU.add,
                        )
                        nc.vector.tensor_tensor(
                            Z[:, hs, :], w1[:, hs, :], u5[:, hs, 4, :], ALU.add
                        )
                        nc.vector.reciprocal_approx_fast(
                            Zr[:, hs, :].rearrange("p h b -> p (h b)"),
                            Z[:, hs, :].rearrange("p h b -> p (h b)"),
                        )
                        if last:
                            # endgame: keep the whole chain on DVE
                            nc.vector.tensor_copy(Zrb[:, hs, :], Zr[:, hs, :])
                            nc.vector.tensor_tensor(
                                cst[:, hs, :, :], e[:, hs, :, :],
                                Zrb[:, hs, :].unsqueeze(2)
                                .broadcast_to((P, QH, J, NB)),
                                ALU.mult,
                            )
                        else:
                            nc.gpsimd.tensor_copy(Zrb[:, hs, :], Zr[:, hs, :])
                            # c = e * Zr -> fp16; j 0:7 DVE, 7:10 Pool
                            nc.vector.tensor_tensor(
                                cst[:, hs, 0:7, :], e[:, hs, 0:7, :],
                                Zrb[:, hs, :].unsqueeze(2)
                                .broadcast_to((P, QH, 7, NB)),
                                ALU.mult,
                            )
                            nc.gpsimd.tensor_mul(
                                cst[:, hs, 7:10, :], e[:, hs, 7:10, :],
                                Zrb[:, hs, :].unsqueeze(2)
                                .broadcast_to((P, QH, 3, NB)),
                            )
                        for h in range(h0, h0 + QH):
                            nc.tensor.matmul(
                                yT_ps[:, :],
                                x_f16[:, h, :],
                                cst[:, h, :, :],
                                start=(h == 0), stop=(h == NH - 1),
                            )

                    ranges = [(0, 16), (16, 16), (32, 16), (48, 8), (56, 8)]
                    emit_logits_exp(*ranges[0])
                    for gi, r in enumerate(ranges):
                        if gi + 1 < len(ranges):
                            emit_logits_exp(*ranges[gi + 1])
                        emit_tree_c_y(*r, last=(gi == len(ranges) - 1))

                    tail(m, yT_ps[:, :])

    nc.compile()
    return nc


_NC_CACHE = None


def _get_nc():
    global _NC_CACHE
    if _NC_CACHE is None:
        _NC_CACHE = _build_nc()
    return _NC_CACHE


def kernel(x, W, bias):
    x = np.asarray(x, dtype=np.float32)
    W = np.ascontiguousarray(np.asarray(W, dtype=np.float32))
    bias = np.ascontiguousarray(np.asarray(bias, dtype=np.float32))
    B = x.shape[0]
    per = B // N_CORES

    nc = _get_nc()
    in_maps = _make_in_maps(x, W, bias)
    res = bass_utils.run_bass_kernel_spmd(
        nc, in_maps, core_ids=list(range(N_CORES))
    )
    # out row is (j, b) squared lengths: [1, 80] -> [b, j], sqrt here
    outs = [np.sqrt(r["out"]).reshape(J, NB).T for r in res.results]
    return np.concatenate(outs, axis=0)


def _make_in_maps(x, W, bias):
    B = x.shape[0]
    per = B // N_CORES
    in_maps = []
    for i in range(N_CORES):
        xc = x[i * per:(i + 1) * per]                       # [8, 8192, 8]
        xa = np.concatenate(
            [xc, np.ones((per, IN, 1), np.float32)], axis=2
        ).astype(np.float16)                                # [8, 8192, 9]
        r = xa.reshape(per, P, NH, DA)
        # xf[k, p, hh, (b,d)] = xa[b, p*NH + k*16 + hh, d]
        xf = np.ascontiguousarray(
            r.transpose(1, 2, 0, 3).reshape(P, 4, NH // 4, KT)
            .transpose(1, 0, 2, 3)
        )
        # xt[k, (b,d), h, p] = xa[b, p*NH + (k*16+h), d]
        xt_full = r.transpose(0, 3, 2, 1).reshape(KT, NH, P)
        xt = np.ascontiguousarray(xt_full.reshape(KT, 4, NH // 4, P)
                                  .transpose(1, 0, 2, 3))
        in_maps.append({"xf": xf, "xt": xt, "W": W, "bias": bias})
    return in_maps


if __name__ == "__main__":
    rng = np.random.default_rng(0)
    x = rng.standard_normal((64, IN, D), dtype=np.float32)
    W = (rng.standard_normal((D, J * KD)) / np.sqrt(D)).astype(np.float32)
    bias = (rng.standard_normal(J * KD) * 0.01).astype(np.float32)
    out = kernel(x=x, W=W, bias=bias)
    print(out.shape, out[0])


# revision 52
# speedup vs baseline: 1.0113x; 1.0113x over previous
"""DenseCapsule routing kernel for Trainium2 (Bass/Tile), 8-core data-parallel.

Problem: x [64, 8192, 8], W [8, 160], bias [160] ->
  x_hat = (x @ W + bias).reshape(64, 8192, 10, 16)
  3 dynamic-routing iterations (softmax over out_num=10, weighted sum over
  in_num=8192, squash over the 10-axis, agreement update), return
  ||outputs||_2 over out_dim -> [64, 10].

Key algebra (x_hat never materialized):
  yT[(b,d), (j,b')] = sum_i x_aug[i,(b,d)] c[i,(j,b')]   (PE, masked by cBLK)
  s8T[k, (j,b)]     = per-j matmuls W_aug vs masked yT    (PE, f32)
  squash runs on the k-partition layout [16, (j,b)] so the vhat matmuls
  need no transposes; vT[d, (j,b)] via per-j matmuls; blkv = mask(cREP@vacc).
  b_logits = xT^T @ blkv, single fp16 blkv (no hi/lo split).
  softmax: exp on ACT (bf16), Z pair-tree on DVE 2x, c = e*Zr -> fp16.

Sharding: batch 64 -> 8 cores x 8 batches. Row space (b,d) = b*9+d (72 rows),
cols (j,b) = j*8+b (80). Output row [1, 80] = lengths at (j,b).
"""

from contextlib import ExitStack

import numpy as np

import concourse.bacc as bacc
import concourse.bass as bass
import concourse.mybir as mybir
import concourse.tile as tile
import concourse.bass_utils as bass_utils

f32 = mybir.dt.float32
bf16 = mybir.dt.bfloat16
fp16 = mybir.dt.float16
AF = mybir.ActivationFunctionType
ALU = mybir.AluOpType

P = 128          # SBUF partitions
NH = 64          # i-chunks per batch (8192 / 128)
NB = 8           # batches per core
D = 8            # input capsule dim
DA = 9           # augmented (+ ones column)
J = 10           # out_num
KD = 16          # out_dim
KT = NB * DA     # 72 rows (b, d)
BJ = NB * J      # 80 cols (j, b) = j*8+b
IN = 8192
N_CORES = 8


def _build_nc():
    nc = bacc.Bacc(
        "TRN2", target_bir_lowering=False, debug=False, num_devices=N_CORES
    )

    xf_d = nc.dram_tensor(
        "xf", [4, P, NH // 4, KT], fp16, kind="ExternalInput"
    ).ap()
    xt_d = nc.dram_tensor(
        "xt", [4, KT, NH // 4, P], fp16, kind="ExternalInput"
    ).ap()
    w_d = nc.dram_tensor("W", [D, J * KD], f32, kind="ExternalInput").ap()
    bias_d = nc.dram_tensor("bias", [J * KD], f32, kind="ExternalInput").ap()
    out_d = nc.dram_tensor("out", [1, BJ], f32, kind="ExternalOutput").ap()

    # ---- structural constants ----
    # cpack cols: 0:80 cBLK (rows 0:72), 80:152 cREP (rows 0:9),
    #             152:161 eye9 (rows 0:9), 161:162 ones column (all rows)
    C_BLK, C_REP, C_E9, C_ONE = 0, 80, 152, 161
    CPW = C_ONE + 1
    cpack_np = np.zeros((P, CPW), dtype=np.float32)
    for b in range(NB):
        for d in range(DA):
            for j in range(J):
                cpack_np[b * DA + d, C_BLK + j * NB + b] = 1.0
    for d in range(DA):
        for b in range(NB):
            cpack_np[d, C_REP + b * DA + d] = 1.0
    cpack_np[0:DA, C_E9:C_E9 + DA] = np.eye(DA, dtype=np.float32)
    cpack_np[:, C_ONE] = 1.0

    cpack_d = nc.inline_tensor(cpack_np, "cpack").ap()

    with tile.TileContext(nc) as tc, ExitStack() as ctx:
        sbp = ctx.enter_context(tc.tile_pool(name="sbp", bufs=1))

        def T(shape, name, dt=f32):
            return sbp.tile(shape, dt, name=name, tag=name)

        # ----- persistent SBUF tensors -----
        x_f16 = T([P, NH, KT], "x_f16", fp16)         # fp16 x_aug (host-cast)
        xT = T([KT, NH, P], "xT", fp16)               # x_aug^T (host-built)
        cpack = T([P, CPW], "cpack")
        cBLK = cpack[0:KT, C_BLK:C_BLK + BJ]
        cREP = cpack[0:DA, C_REP:C_REP + KT]
        eye9 = cpack[0:DA, C_E9:C_E9 + DA]
        onesF = cpack[:, C_ONE:C_ONE + 1]

        W10flat = T([DA, J * KD], "W10flat")          # W_aug rows d
        WBIGall = T([KT, J * KD], "WBIGall")          # W_aug repl. over b
        WT10 = T([KD, J, DA], "WT10")                 # W_aug^T per j
        part0 = T([P, 4, KT], "part0")                # m0 f32 colsum partials
        y0row = T([1, KT], "y0row")                   # m0 colsum row
        tenth80 = T([1, BJ], "tenth80")               # 0.1 expander row
        phalf16 = T([KD, NB], "phalf16")              # +0.5 (pow -> sqrt)
        halfrow = T([1, BJ], "halfrow")               # +0.5 (pow warm)
        vaccT = T([DA, BJ], "vaccT")                  # accumulated vhat^T
        blkv_t = [T([KT, BJ], f"blkv{m}", fp16) for m in range(2)]
        lsum = T([1, BJ], "lsum")
        powwarm = T([1, 1], "powwarm")

        e_st = [None, T([P, NH, J, NB], "e1", bf16), T([P, NH, J, NB], "e2", bf16)]
        c_st = [None, T([P, NH, J, NB], "c1", fp16), T([P, NH, J, NB], "c2", fp16)]
        u5_t = [None, T([P, NH, 5, NB], "u5_1", bf16), T([P, NH, 5, NB], "u5_2", bf16)]
        v2_t = [None, T([P, NH, 2, NB], "v2_1", bf16), T([P, NH, 2, NB], "v2_2", bf16)]
        w1_t = [None, T([P, NH, NB], "w1_1", bf16), T([P, NH, NB], "w1_2", bf16)]
        Z_t = [None, T([P, NH, NB], "Z_1"), T([P, NH, NB], "Z_2")]
        Zr_t = [None, T([P, NH, NB], "Zr_1"), T([P, NH, NB], "Zr_2")]
        Zrb_t = [None, T([P, NH, NB], "Zrb_1", bf16), T([P, NH, NB], "Zrb_2", bf16)]

        # per-m tail tensors
        yTm_t = [T([KT, BJ], f"yTm_{m}") for m in range(3)]
        s2T_t = [T([KD, J + 1, NB], f"s2T_{m}") for m in range(3)]
        nsq_t = [T([KD, NB], f"nsq_{m}") for m in range(3)]
        uin_t = [T([KD, NB], f"uin_{m}") for m in range(3)]
        wp1_t = [T([KD, NB], f"wp1_{m}") for m in range(3)]
        r1_t = [T([KD, NB], f"r1_{m}") for m in range(3)]
        sc_t = [T([KD, NB], f"sc_{m}") for m in range(3)]
        o8T_t = [T([KD, J, NB], f"o8T_{m}") for m in range(3)]
        osqT = T([KD, J, NB], "osqT")

        # ----- inputs: xf pieces first on sync HWDGE (feeds colsums);
        # consts + xt split over scalar HWDGE and gpsimd SWDGE ----------
        for k in range(4):
            nc.sync.dma_start(
                x_f16[:, k * (NH // 4):(k + 1) * (NH // 4), :], xf_d[k]
            )
        nc.scalar.dma_start(cpack[:, :], cpack_d[:, :])
        nc.scalar.dma_start(W10flat[0:D, :], w_d[:, :])
        nc.scalar.dma_start(
            W10flat[D:DA, :],
            bass.AP(tensor=bias_d.tensor, offset=0,
                    ap=[[J * KD, 1], [1, J * KD]]),
        )
        for k in (0, 1):
            nc.scalar.dma_start(
                xT[:, k * (NH // 4):(k + 1) * (NH // 4), :], xt_d[k]
            )
        for k in (2, 3):
            nc.gpsimd.dma_start(
                xT[:, k * (NH // 4):(k + 1) * (NH // 4), :], xt_d[k]
            )

        # tiny memsets on gpsimd; pad memset on DVE (runs during first DMAs)
        nc.gpsimd.memset(phalf16[:, :], 0.5)
        nc.gpsimd.memset(halfrow[:, :], 0.5)
        nc.gpsimd.memset(tenth80[:, :], 1.0 / J)
        for m in range(3):
            nc.gpsimd.memset(s2T_t[m][:, J, :], 1e-12)
        # warm the gpsimd pow library off the critical chain
        nc.gpsimd.tensor_tensor(
            powwarm[:, :], halfrow[0:1, 0:1], halfrow[0:1, 0:1], ALU.pow
        )

        # ----- m0 colsums (DVE): per-piece partials over h, then merge ----
        for k in range(4):
            nc.vector.reduce_sum(
                part0[:, k, :],
                x_f16[:, k * (NH // 4):(k + 1) * (NH // 4), :]
                .transpose([0, 2, 1]),
                axis=mybir.AxisListType.X,
            )

        with tc.tile_pool(name="wpp", bufs=1, space="PSUM") as wpp:
            # ---- W prep: WBIGall + WT10 (early, overlaps x DMA) ----
            wb_ps = wpp.tile([KT, J * KD], f32, tag="wb", name="wb_ps")
            nc.tensor.matmul(
                wb_ps[:, :], cREP, W10flat[:, :], start=True, stop=True
            )
            nc.scalar.copy(WBIGall[:, :], wb_ps[:, :])
            wt_ps = wpp.tile([KD, J, DA], f32, tag="wt", name="wt_ps")
            for j in range(J):
                nc.tensor.transpose(
                    wt_ps[:, j, :], W10flat[:, j * KD:(j + 1) * KD], eye9
                )
            nc.scalar.copy(WT10[:, :, :], wt_ps[:, :, :])



        with tc.tile_pool(name="ypp", bufs=1, space="PSUM") as ypp:

            def tail(m, yT72, fill=()):
                """mask -> s-MMs -> squash (k-layout) -> v-MMs -> blkv.

                fill: callbacks emitting PE work injected after the s-MM /
                v-MM stages so PE stays busy during the DVE scalar chain.
                """
                yTm, s2T = yTm_t[m], s2T_t[m]
                nsqT, u, wp1, r1, scT = (
                    nsq_t[m], uin_t[m], wp1_t[m], r1_t[m], sc_t[m]
                )
                o8T = o8T_t[m]
                # mask: yTm[(b,d),(j,b')] = yT * (b==b')
                nc.vector.tensor_tensor(yTm[:, :], yT72, cBLK, ALU.mult)
                s8_ps = ypp.tile([KD, J, NB], f32, tag="ysm", name=f"s8_{m}")
                for j in range(J):
                    nc.tensor.matmul(
                        s8_ps[:, j, :],
                        WBIGall[:, j * KD:(j + 1) * KD],
                        yTm[:, j * NB:(j + 1) * NB],
                        start=True, stop=True,
                    )

                # squash scalars on [16, 8] (k-partition layout);
                # eps lives in s2T's 11th j-column (prologue memset)
                nc.scalar.activation(s2T[:, 0:J, :], s8_ps[:, :, :], AF.Square)
                nc.vector.reduce_sum(
                    nsqT[:, :],
                    s2T[:, :, :].transpose([0, 2, 1]),
                    axis=mybir.AxisListType.X,
                )
                # sc = sqrt(nsq)/(1+nsq): pow on Pool overlaps the DVE recip
                nc.gpsimd.tensor_tensor(
                    u[:, :], nsqT[:, :], phalf16[:, :], ALU.pow
                )
                nc.vector.tensor_scalar_add(wp1[:, :], nsqT[:, :], 1.0)
                nc.vector.reciprocal_approx_fast(r1[:, :], wp1[:, :])
                if len(fill) > 0:
                    fill[0]()
                nc.vector.tensor_tensor(scT[:, :], u[:, :], r1[:, :], ALU.mult)
                if m == 2:
                    # final lengths^2: osq = s^2 * sc^2 reuses s2T; ||.||_k
                    # summed across k-partitions by the ones-matmul
                    nc.vector.tensor_tensor(
                        wp1[:, :], scT[:, :], scT[:, :], ALU.mult
                    )
                    nc.vector.tensor_tensor(
                        osqT[:, :, :],
                        s2T[:, 0:J, :],
                        wp1[:, :].unsqueeze(1).broadcast_to((KD, J, NB)),
                        ALU.mult,
                    )
                    ls_ps = ypp.tile([1, BJ], f32, tag="ysm", name="ls_ps")
                    nc.tensor.matmul(
                        ls_ps[:, :], onesF[0:KD, :],
                        osqT[:, :, :].rearrange("p j b -> p (j b)"),
                        start=True, stop=True,
                    )
                    nc.vector.tensor_copy(lsum[:, :], ls_ps[:, :])
                    nc.sync.dma_start(out_d[:, :], lsum[:, :])
                    return
                # o8T = s8 * sc, straight from PSUM (sc broadcast over j)
                nc.vector.tensor_tensor(
                    o8T[:, :, :],
                    s8_ps[:, :, :],
                    scT[:, :].unsqueeze(1).broadcast_to((KD, J, NB)),
                    ALU.mult,
                )
                # vhat: vT[d,(j,b)] via per-j matmuls; accumulate; expand+mask
                vT_ps = ypp.tile([DA, BJ], f32, tag="ysm", name=f"vT_{m}")
                for j in range(J):
                    nc.tensor.matmul(
                        vT_ps[:, j * NB:(j + 1) * NB],
                        WT10[:, j, :],
                        o8T[:, j, :],
                        start=True, stop=True,
                    )
                if len(fill) > 1:
                    fill[1]()
                if m == 0:
                    nc.vector.tensor_copy(vaccT[:, :], vT_ps[:, :])
                else:
                    nc.vector.tensor_tensor(
                        vaccT[:, :], vaccT[:, :], vT_ps[:, :], ALU.add
                    )
                vd_ps = ypp.tile([KT, BJ], f32, tag="ysm", name=f"vd_{m}")
                nc.tensor.matmul(
                    vd_ps[:, :], cREP, vaccT[:, :], start=True, stop=True
                )
                nc.vector.tensor_tensor(
                    blkv_t[m][:, :], vd_ps[:, :], cBLK, ALU.mult
                )

            # ================= m = 0 (uniform c shortcut, f32) =================
            # accumulate the 4 piece-partials in PSUM via 4 tiny matmuls
            y0r_ps = ypp.tile([1, KT], f32, tag="ysm", name="y0r")
            for k in range(4):
                nc.tensor.matmul(
                    y0r_ps[:, :], onesF, part0[:, k, :],
                    start=(k == 0), stop=(k == 3),
                )
            nc.vector.tensor_copy(y0row[:, :], y0r_ps[:, :])
            yT0_ps = ypp.tile([KT, BJ], f32, tag="ysm", name="yT0")
            nc.tensor.matmul(
                yT0_ps[:, :], y0row[:, :], tenth80[:, :], start=True, stop=True
            )

            tail(0, yT0_ps[:, :])

            # ================= m = 1, 2 =================
            with tc.tile_pool(name="bwp", bufs=3, space="PSUM") as bwp:
                for m in (1, 2):
                    blkv = blkv_t[m - 1]
                    e = e_st[m]
                    cst = c_st[m]
                    u5, v2, w1 = u5_t[m], v2_t[m], w1_t[m]
                    Z, Zr, Zrb = Z_t[m], Zr_t[m], Zrb_t[m]

                    yT_ps = ypp.tile([KT, BJ], f32, tag="ysm", name=f"yT_{m}")

                    def emit_logits_exp(h0, QH):
                        # waves of 8 chunks; exp per wave
                        for w in range(QH // 8):
                            hw0 = h0 + w * 8
                            bw = bwp.tile(
                                [P, 2, 512], f32, tag="bw",
                                name=f"bw_{m}_{hw0}",
                            )
                            for c8 in range(8):
                                h = hw0 + c8
                                off = (c8 % 4) * BJ
                                nc.tensor.matmul(
                                    bw[:, c8 // 4, off:off + BJ],
                                    xT[:, h, :], blkv[:, :],
                                    start=True, stop=True,
                                )
                            nc.scalar.activation(
                                e[:, hw0:hw0 + 8, :, :]
                                .rearrange("p (a c) j b -> p a c (j b)", a=2),
                                bw[:, :, 0:4 * BJ]
                                .rearrange("p a (c x) -> p a c x", x=BJ),
                                AF.Exp,
                            )

                    def emit_tree_c_y(h0, QH, last=False):
                        hs = slice(h0, h0 + QH)
                        nc.vector.tensor_tensor(
                            u5[:, hs, :, :], e[:, hs, 0:5, :], e[:, hs, 5:10, :],
                            ALU.add,
                        )
                        nc.vector.tensor_tensor(
                            v2[:, hs, :, :], u5[:, hs, 0:2, :], u5[:, hs, 2:4, :],
                            ALU.add,
                        )
                        nc.vector.tensor_tensor(
                            w1[:, hs, :], v2[:, hs, 0, :], v2[:, hs, 1, :],
                            ALU.add,
                        )
                        nc.vector.tensor_tensor(
                            Z[:, hs, :], w1[:, hs, :], u5[:, hs, 4, :], ALU.add
                        )
                        nc.vector.reciprocal_approx_fast(
                            Zr[:, hs, :].rearrange("p h b -> p (h b)"),
                            Z[:, hs, :].rearrange("p h b -> p (h b)"),
                        )
                        if last:
                            # endgame: keep the whole chain on DVE
                            nc.vector.tensor_copy(Zrb[:, hs, :], Zr[:, hs, :])
                            nc.vector.tensor_tensor(
                                cst[:, hs, :, :], e[:, hs, :, :],
                                Zrb[:, hs, :].unsqueeze(2)
                                .broadcast_to((P, QH, J, NB)),
                                ALU.mult,
                            )
                        else:
                            nc.gpsimd.tensor_copy(Zrb[:, hs, :], Zr[:, hs, :])
                            # c = e * Zr -> fp16; j 0:7 DVE, 7:10 Pool
                            nc.vector.tensor_tensor(
                                cst[:, hs, 0:7, :], e[:, hs, 0:7, :],
                                Zrb[:, hs, :].unsqueeze(2)
                                .broadcast_to((P, QH, 7, NB)),
                                ALU.mult,
                            )
                            nc.gpsimd.tensor_mul(
                                cst[:, hs, 7:10, :], e[:, hs, 7:10, :],
                                Zrb[:, hs, :].unsqueeze(2)
                                .broadcast_to((P, QH, 3, NB)),
                            )
                        for h in range(h0, h0 + QH):
                            nc.tensor.matmul(
                                yT_ps[:, :],
                                x_f16[:, h, :],
                                cst[:, h, :, :],
                                start=(h == 0), stop=(h == NH - 1),
                            )

                    ranges = [(0, 16), (16, 16), (32, 16), (48, 8), (56, 8)]
                    emit_logits_exp(*ranges[0])
                    for gi, r in enumerate(ranges):
                        if gi + 1 < len(ranges):
                            emit_logits_exp(*ranges[gi + 1])
                        emit_tree_c_y(*r, last=(gi == len(ranges) - 1))

                    tail(m, yT_ps[:, :])

    nc.compile()
    return nc


_NC_CACHE = None


def _get_nc():
    global _NC_CACHE
    if _NC_CACHE is None:
        _NC_CACHE = _build_nc()
    return _NC_CACHE


def kernel(x, W, bias):
    x = np.asarray(x, dtype=np.float32)
    W = np.ascontiguousarray(np.asarray(W, dtype=np.float32))
    bias = np.ascontiguousarray(np.asarray(bias, dtype=np.float32))
    B = x.shape[0]
    per = B // N_CORES

    nc = _get_nc()
    in_maps = _make_in_maps(x, W, bias)
    res = bass_utils.run_bass_kernel_spmd(
        nc, in_maps, core_ids=list(range(N_CORES))
    )
    # out row is (j, b) squared lengths: [1, 80] -> [b, j], sqrt here
    outs = [np.sqrt(r["out"]).reshape(J, NB).T for r in res.results]
    return np.concatenate(outs, axis=0)


def _make_in_maps(x, W, bias):
    B = x.shape[0]
    per = B // N_CORES
    in_maps = []
    for i in range(N_CORES):
        xc = x[i * per:(i + 1) * per]                       # [8, 8192, 8]
        xa = np.concatenate(
            [xc, np.ones((per, IN, 1), np.float32)], axis=2
        ).astype(np.float16)                                # [8, 8192, 9]
        r = xa.reshape(per, P, NH, DA)
        # xf[k, p, hh, (b,d)] = xa[b, p*NH + k*16 + hh, d]
        xf = np.ascontiguousarray(
            r.transpose(1, 2, 0, 3).reshape(P, 4, NH // 4, KT)
            .transpose(1, 0, 2, 3)
        )
        # xt[k, (b,d), h, p] = xa[b, p*NH + (k*16+h), d]
        xt_full = r.transpose(0, 3, 2, 1).reshape(KT, NH, P)
        xt = np.ascontiguousarray(xt_full.reshape(KT, 4, NH // 4, P)
                                  .transpose(1, 0, 2, 3))
        in_maps.append({"xf": xf, "xt": xt, "W": W, "bias": bias})
    return in_maps


if __name__ == "__main__":
    rng = np.random.default_rng(0)
    x = rng.standard_normal((64, IN, D), dtype=np.float32)
    W = (rng.standard_normal((D, J * KD)) / np.sqrt(D)).astype(np.float32)
    bias = (rng.standard_normal(J * KD) * 0.01).astype(np.float32)
    out = kernel(x=x, W=W, bias=bias)
    print(out.shape, out[0])
